# revision 30
# baseline (speedup 1.0000x reference)
"""Multi-head attention (RMSNorm-QK + RoPE) Trainium2 Bass kernel — v2.

Sharding: 8 cores = 4 batches x 2 head-groups (6 heads each).
Host sums the two partial y's per batch and adds proj bias.

v2 design (vs baseline):
  - qkv GEMM in bf16 (same PE rate, FWL weight loads, half DMA).
  - RMSNorm commutes with RoPE (rotation preserves pair norms): rope runs on
    raw q/k, the rsqrt factor multiplies afterwards, q & k fused per tile.
  - Q/K transposes via DMA xbar (dma_start_transpose) instead of PE+DVE.
  - Phase 2 is one software-pipelined stream over all (qc, pp) chunks:
    QK(c+1) is emitted before PV(c) so the scalar-engine exp stream never
    stalls; PSUM: 2x3-bank QK slabs + 2x1-bank PV accumulators.
  - Per-chunk exp = one ACTIVATE over [128, 3, 512] PSUM.
  - Softmax denominators ride as the 65th V column; reciprocal runs on a
    [128, 24] repacked tile (DMA bounce) instead of [1, 512] strips.
  - Projection/broadcast work for chunk qc is interleaved into qc+1's
    attention stream in small pieces to keep the exp stream saturated.
"""

import sys

for _p in ("/opt/trn_rl_repo", "/root/.axon_site/_ro/trn_rl_repo"):
    if _p not in sys.path:
        sys.path.insert(0, _p)

import numpy as np
import ml_dtypes

import bass_rust
import concourse.bass as bass
import concourse.mybir as mybir
import concourse.tile as tile
from concourse.bass_utils import run_bass_kernel_spmd

# Problem constants (hardcoded per contract)
B, N, D = 4, 2048, 768
H, HD = 12, 64
HPC = 6              # heads per core
NT = N // 128        # 16 seq tiles
EPS = 1e-6
THETA = 10000.0
SCALE = HD ** -0.5   # 0.125

F32 = mybir.dt.float32
F32R = mybir.dt.float32r
BF16 = mybir.dt.bfloat16

KERNEL_TRACE = False
_CACHE = {}


# ---------------------------------------------------------------- wait split
_ctr = [0]


def _mk_nop(engine, waits=None, updates=None):
    _ctr[0] += 1
    si = mybir.SyncInfo(on_wait=waits or [], on_update=updates or [])
    return bass_rust.InstNoOp(
        name=f"I-waitfix-{_ctr[0]}", engine=engine, ins=[], outs=[], sync_info=si
    )


def split_multi_waits(nc):
    """This walrus build accepts only ONE sync wait/update per instruction;
    hoist extras onto adjacent same-engine NoOp carriers."""
    for fn in nc.m.functions:
        for bb in fn.blocks:
            insts = bb.instructions
            out = []
            changed = False
            for inst in insts:
                si = inst.sync_info
                if si is None:
                    out.append(inst)
                    continue
                waits = list(si.on_wait or [])
                updates = list(si.on_update or [])
                pre, post = [], []
                if len(waits) > 1:
                    for w in waits[:-1]:
                        pre.append(_mk_nop(inst.engine, waits=[w]))
                    si.on_wait = [waits[-1]]
                    changed = True
                if len(updates) > 1:
                    if inst.opcode == "DMACopy":
                        raise RuntimeError(
                            f"DMACopy {inst.name} has {len(updates)} updates"
                        )
                    for u in updates[1:]:
                        post.append(_mk_nop(inst.engine, updates=[u]))
                    si.on_update = [updates[0]]
                    changed = True
                out.extend(pre)
                out.append(inst)
                out.extend(post)
            if changed:
                insts[:] = out
    return nc


# ---------------------------------------------------------------- host utils
def _rope_tables(norm_w: np.ndarray):
    """cosw[n,d] = cos[n,d]*w[d];  sinw folds the rotate-half sign+swap of w:
    q' = qn*cosw + shuffle32(qn)*sinw  (shuffle32 = swap halves, no negation)."""
    inv_freq = 1.0 / (THETA ** (np.arange(0, HD, 2, dtype=np.float32) / HD))
    t = np.arange(N, dtype=np.float32)
    freqs = np.einsum("i,j->ij", t, inv_freq).astype(np.float32)
    emb = np.concatenate([freqs, freqs], axis=-1)  # [N, HD]
    cos = np.cos(emb).astype(np.float32)
    sin = np.sin(emb).astype(np.float32)
    w = norm_w.astype(np.float32)
    cosw = cos * w[None, :]
    sinw = np.empty_like(sin)
    h = HD // 2
    sinw[:, :h] = -sin[:, :h] * w[None, h:]
    sinw[:, h:] = sin[:, h:] * w[None, :h]
    return cosw, sinw


# ---------------------------------------------------------------- bass build
def build_nc(use_bias: bool):
    FC = 7 if use_bias else 6  # feature chunks of 128 (7th = bias row)
    nc = bass.Bass()

    xt_d = nc.dram_tensor("xt", [FC * 128, N], BF16, kind="ExternalInput")
    wq_d = nc.dram_tensor("wq", [FC * 128, HPC * HD], BF16, kind="ExternalInput")
    wk_d = nc.dram_tensor("wk", [FC * 128, HPC * HD], BF16, kind="ExternalInput")
    wv_d = nc.dram_tensor("wv", [FC * 128, HPC * HD], BF16, kind="ExternalInput")
    wo_d = nc.dram_tensor("wo", [HPC * HD, D], BF16, kind="ExternalInput")
    # rope tables: [:, 0, :] = q variant, [:, 1, :] = k variant (norm_w folded)
    cos_d = nc.dram_tensor("cost", [N, 2 * HD], BF16, kind="ExternalInput")
    sin_d = nc.dram_tensor("sint", [N, 2 * HD], BF16, kind="ExternalInput")
    y_d = nc.dram_tensor("y", [N, D], F32, kind="ExternalOutput")

    with tile.TileContext(nc) as tc:
        with (
            tc.tile_pool(name="const", bufs=1) as constp,
            tc.tile_pool(name="wts", bufs=1) as wts,
            tc.tile_pool(name="persist", bufs=1) as persist,
            tc.tile_pool(name="rope", bufs=2) as rope,
            tc.tile_pool(name="xtile", bufs=3) as xtile,
            tc.tile_pool(name="ptp", bufs=3) as ptp,
            tc.tile_pool(name="otunp", bufs=2) as otunp,
            tc.tile_pool(name="otqp", bufs=2) as otqp,
            tc.tile_pool(name="denp", bufs=2) as denp,
            tc.tile_pool(name="yout", bufs=2) as yout,
            tc.tile_pool(name="work", bufs=2, space="PSUM") as workp,
            tc.tile_pool(name="otp", bufs=2, space="PSUM") as otp,
        ):
            # ---- constants / weights
            ones_sb = constp.tile([128, 64], BF16)
            nc.vector.memset(ones_sb[:, :], 1.0)
            eps_t = constp.tile([128, 1], F32)
            nc.vector.memset(eps_t[:, :], EPS)
            zero_t = constp.tile([128, 1], F32)
            nc.vector.memset(zero_t[:, :], 0.0)

            # Weights chunked per feature-block so the first qkv matmuls can
            # start after ~3 small DMAs instead of waiting for 3.5 MB.
            xt_pre = {}

            def load_xt(i):
                t = xtile.tile([128, FC, 128], BF16, tag="xt", name=f"xt{i}")
                nc.sync.dma_start(
                    t[:, :, :],
                    xt_d.rearrange("(c p) n -> p c n", p=128)[:, :, i * 128:(i + 1) * 128],
                )
                return t

            xt_pre[0] = load_xt(0)
            xt_pre[1] = load_xt(1)
            w_sbs = []
            for wd, nm in ((wq_d, "wq"), (wk_d, "wk"), (wv_d, "wv")):
                wsb = wts.tile([128, FC, HPC * HD], BF16, tag=nm, name=nm)
                w_sbs.append(wsb)
            for c in range(FC):
                for t, wd in enumerate((wq_d, wk_d, wv_d)):
                    nc.sync.dma_start(
                        w_sbs[t][:, c, :],
                        wd.rearrange("(c p) n -> p c n", p=128)[:, c, :],
                    )
            # rope tables (bf16 host-side)
            cos_sb = constp.tile([128, NT, 2, HD], BF16, tag="cos")
            nc.sync.dma_start(
                cos_sb[:, :, :, :],
                cos_d.rearrange("(t p) (a d) -> p t a d", p=128, a=2),
            )
            sin_sb = constp.tile([128, NT, 2, HD], BF16, tag="sin")
            nc.sync.dma_start(
                sin_sb[:, :, :, :],
                sin_d.rearrange("(t p) (a d) -> p t a d", p=128, a=2),
            )
            wo_sb = wts.tile([128, 3, D], BF16, tag="wo")
            nc.sync.dma_start(wo_sb[:, :, :], wo_d.rearrange("(c p) n -> p c n", p=128))

            qt_sb = persist.tile([128, 3, N], BF16, tag="qt")
            kt_sb = persist.tile([128, 3, N], BF16, tag="kt")
            vaug = persist.tile([128, NT, HPC, 65], BF16, tag="vaug")
            nc.vector.memset(vaug[:, :, :, 64:65], 1.0)

            # ================= phase 1: qkv + (rope, then rmsnorm-scale) + T
            # Stage B (ln/exp/rs-mul/transposes) of tile i is emitted during
            # tile i+1 so the ACT queue never head-of-line blocks on the DVE
            # reduce of the same tile.
            def ph1_stage_b(p):
                i, ss, c2 = p
                lg = rope.tile([128, 2, HPC], F32, tag="lg")
                nc.scalar.activation(lg[:, :, :], ss[:, :, :],
                                     mybir.ActivationFunctionType.Ln,
                                     bias=eps_t[:, :], scale=1.0 / HD)
                rs = rope.tile([128, 2, HPC], BF16, tag="rs")
                nc.scalar.activation(rs[:, :, :], lg[:, :, :],
                                     mybir.ActivationFunctionType.Exp,
                                     bias=zero_t[:, :], scale=-0.5)
                ro = rope.tile([128, 2, HPC, HD], BF16, tag="ro")
                nc.vector.tensor_mul(
                    ro[:, :, :, :], c2[:, :, :, :],
                    rs[:, :, :, None].to_broadcast((128, 2, HPC, HD)),
                )
                rof = ro.rearrange("p a h d -> p (a h d)")
                nc.sync.dma_start_transpose(
                    qt_sb[:, 0:3, i * 128:(i + 1) * 128], rof[:, 0:384])
                nc.sync.dma_start_transpose(
                    kt_sb[:, 0:3, i * 128:(i + 1) * 128], rof[:, 384:768])

            ph1_pend = None
            for i in range(NT):
                xt_sb = xt_pre.pop(i) if i in xt_pre else load_xt(i)
                ps = workp.tile([128, 3, 512], F32, tag="work")
                for c in range(FC):
                    for t in range(3):
                        nc.tensor.matmul(
                            ps[:, t, 0:384],
                            xt_sb[:, c, :],
                            w_sbs[t][:, c, :],
                            start=(c == 0),
                            stop=(c == FC - 1),
                        )
                # V -> vaug (scalar engine, PSUM->SBUF cast copy)
                nc.scalar.copy(
                    vaug[:, i, :, 0:64],
                    ps[:, 2, 0:384].rearrange("p (h d) -> p h d", h=HPC),
                )
                # q|k fused [128, 2, 6, 64] bf16
                qk = rope.tile([128, 2, HPC, HD], BF16, tag="qk")
                nc.scalar.copy(
                    qk[:, :, :, :],
                    ps[:, 0:2, 0:384].rearrange("p a (h d) -> p a h d", h=HPC),
                )
                sq = rope.tile([128, 2, HPC, HD], BF16, tag="sq")
                nc.scalar.activation(
                    sq[:, :, :, :],
                    ps[:, 0:2, 0:384].rearrange("p a (h d) -> p a h d", h=HPC),
                    mybir.ActivationFunctionType.Square,
                    bias=zero_t[:, :],
                )
                ss = rope.tile([128, 2, HPC], F32, tag="ss")
                nc.vector.reduce_sum(ss[:, :, :], sq[:, :, :, :], axis=mybir.AxisListType.X)

                cosb = cos_sb[:, i, :, None, :].to_broadcast((128, 2, HPC, HD))
                sinb = sin_sb[:, i, :, None, :]
                a = rope.tile([128, 2, HPC, HD], BF16, tag="a")
                nc.vector.tensor_mul(a[:, :, :, :], qk[:, :, :, :], cosb)
                bt = rope.tile([128, 2, HPC, HD], BF16, tag="bt")
                h = HD // 2
                nc.vector.tensor_mul(
                    bt[:, :, :, 0:h], qk[:, :, :, h:HD],
                    sinb[:, :, :, 0:h].to_broadcast((128, 2, HPC, h)),
                )
                nc.vector.tensor_mul(
                    bt[:, :, :, h:HD], qk[:, :, :, 0:h],
                    sinb[:, :, :, h:HD].to_broadcast((128, 2, HPC, h)),
                )
                c2 = rope.tile([128, 2, HPC, HD], BF16, tag="c2")
                nc.vector.tensor_add(c2[:, :, :, :], a[:, :, :, :], bt[:, :, :, :])
                if ph1_pend is not None:
                    ph1_stage_b(ph1_pend)
                ph1_pend = (i, ss, c2)
            ph1_stage_b(ph1_pend)

            # ================= phase 2: one pipelined stream over (qc, pp)
            units = []
            for qc in range(4):
                for pp in range(3):
                    fills = [(kt, hh) for kt in range(NT) for hh in range(2)]
                    chunks = [fills[j:j + 3] for j in range(0, 32, 3)]
                    for ci, ch in enumerate(chunks):
                        units.append((qc, pp, ch, ci == len(chunks) - 1))

            cur_ots = {}
            otun_by_qc = {}
            den_tiles = {}
            otq_by_qc = {}
            finish_pieces = []  # queue of callables, popped one per unit
            pend = None         # (qc, pp, ch, pt, last)

            def emit_den(qc, h0, h1):
                # 1/den = exp(-ln(den)) on the scalar engine: den rows live on
                # one partition, so DVE reciprocal (8 cyc/elem) or a DMA
                # repack round-trip are both far worse than 2 ACT passes.
                otun_all = otun_by_qc[qc]
                if qc not in den_tiles:
                    lgd = denp.tile([65, HPC, 512], F32, tag="lgd",
                                    name=f"lgd{qc}")
                    rec = denp.tile([65, HPC, 512], BF16, tag="rec",
                                    name=f"rec{qc}")
                    den_tiles[qc] = (lgd, rec)
                lgd, rec = den_tiles[qc]
                nc.scalar.activation(lgd[64:65, h0:h1, :],
                                     otun_all[64:65, h0:h1, :],
                                     mybir.ActivationFunctionType.Ln,
                                     bias=zero_t[64:65, :], scale=1.0)
                nc.scalar.activation(rec[64:65, h0:h1, :],
                                     lgd[64:65, h0:h1, :],
                                     mybir.ActivationFunctionType.Exp,
                                     bias=zero_t[64:65, :], scale=-1.0)

            def norm_piece(qc, h0, h1):
                def fn():
                    otun_all = otun_by_qc[qc]
                    rec = den_tiles[qc][1]
                    if qc not in otq_by_qc:
                        otq_by_qc[qc] = otqp.tile([128, 3, 512], BF16,
                                                  tag="otq", name=f"otq{qc}")
                    otq = otq_by_qc[qc]
                    bcw = workp.tile([128, 3, 512], F32, tag="work",
                                     name="bcw")
                    for j, hloc in enumerate(range(h0, h1)):
                        pp_, hh_ = hloc // 2, hloc % 2
                        nc.tensor.matmul(bcw[0:64, j, :], ones_sb[64:65, :],
                                         rec[64:65, hloc, :],
                                         start=True, stop=True)
                        nc.vector.tensor_mul(
                            otq[hh_ * 64:(hh_ + 1) * 64, pp_, :],
                            otun_all[0:64, hloc, :],
                            bcw[0:64, j, :],
                        )
                return fn

            def proj_piece(qc, t0, t1):
                def fn():
                    otq = otq_by_qc[qc]
                    for qt4 in range(t0, t1):
                        q0 = qc * 512 + qt4 * 128
                        yps = workp.tile([128, 3, 512], F32, tag="work",
                                         name="yps")
                        for c in range(3):
                            nc.tensor.matmul(
                                yps[:, 0, :],
                                otq[:, c, qt4 * 128:(qt4 + 1) * 128],
                                wo_sb[:, c, 0:512],
                                start=(c == 0), stop=(c == 2),
                            )
                        for c in range(3):
                            nc.tensor.matmul(
                                yps[:, 1, 0:256],
                                otq[:, c, qt4 * 128:(qt4 + 1) * 128],
                                wo_sb[:, c, 512:768],
                                start=(c == 0), stop=(c == 2),
                            )
                        ysb = yout.tile([128, D], F32, tag="ysb")
                        nc.vector.tensor_copy(ysb[:, 0:512], yps[:, 0, :])
                        nc.vector.tensor_copy(ysb[:, 512:768], yps[:, 1, 0:256])
                        nc.sync.dma_start(y_d[q0:q0 + 128, :], ysb[:, :])
                return fn

            def flush_pv(p):
                qc, pp, ch, pt, last = p
                key = (qc, pp)
                if key not in cur_ots:
                    cur_ots[key] = [
                        otp.tile([128, 512], F32, tag="ot", name=f"ots{hh}")
                        for hh in range(2)
                    ]
                ots = cur_ots[key]
                for j, (kt, hh) in enumerate(ch):
                    nc.tensor.matmul(
                        ots[hh][0:65, :],
                        vaug[:, kt, pp * 2 + hh, :],
                        pt[:, j, :],
                        start=(kt == 0), stop=(kt == NT - 1),
                    )
                if last:
                    if qc not in otun_by_qc:
                        otun_by_qc[qc] = otunp.tile(
                            [65, HPC, 512], F32, tag="otun", name=f"otun{qc}")
                    otun_all = otun_by_qc[qc]
                    for hh in range(2):
                        nc.vector.tensor_copy(
                            otun_all[0:65, pp * 2 + hh, :], ots[hh][0:65, :])
                    del cur_ots[key]
                    if qc == 3:
                        # Last qc: den + normalize per pp so the final
                        # projection isn't one serial tail.
                        emit_den(3, pp * 2, pp * 2 + 2)
                        norm_piece(3, pp * 2, pp * 2 + 2)()
                    elif pp == 2:
                        emit_den(qc, 0, HPC)
                    if pp == 0 and qc > 0:
                        # bc tiles live in the workp pool (rotating with the
                        # QK slabs), so every piece can be deferred and
                        # spread across units without slot deadlocks.
                        finish_pieces.append(norm_piece(qc - 1, 0, 2))
                        finish_pieces.append(norm_piece(qc - 1, 2, 4))
                        finish_pieces.append(norm_piece(qc - 1, 4, 6))
                        for t in range(4):
                            finish_pieces.append(proj_piece(qc - 1, t, t + 1))

            for (qc, pp, ch, last) in units:
                slab = workp.tile([128, 3, 512], F32, tag="work", name="slab")
                for j, (kt, hh) in enumerate(ch):
                    nc.tensor.matmul(
                        slab[:, j, :],
                        kt_sb[hh * 64:(hh + 1) * 64, pp, kt * 128:(kt + 1) * 128],
                        qt_sb[hh * 64:(hh + 1) * 64, pp, qc * 512:(qc + 1) * 512],
                        start=True, stop=True,
                    )
                pt = ptp.tile([128, 3, 512], BF16, tag="pt")
                nj = len(ch)
                nc.scalar.activation(
                    pt[:, 0:nj, :], slab[:, 0:nj, :],
                    mybir.ActivationFunctionType.Exp,
                    bias=zero_t[:, :], scale=SCALE,
                )
                if pend is not None:
                    flush_pv(pend)
                    if finish_pieces:
                        finish_pieces.pop(0)()
                pend = (qc, pp, ch, pt, last)
            flush_pv(pend)
            for t in range(4):
                proj_piece(3, t, t + 1)()
            while finish_pieces:
                finish_pieces.pop(0)()

    split_multi_waits(nc)
    return nc


# ---------------------------------------------------------------- entry
def kernel(x, qkv_w, qkv_b, proj_w, proj_b, q_norm_w, k_norm_w, _trace=False,
           _debug=False):
    x = np.asarray(x, dtype=np.float32)
    qkv_w = np.asarray(qkv_w, dtype=np.float32)
    qkv_b = np.asarray(qkv_b, dtype=np.float32)
    proj_w = np.asarray(proj_w, dtype=np.float32)
    proj_b = np.asarray(proj_b, dtype=np.float32)
    q_norm_w = np.asarray(q_norm_w, dtype=np.float32)
    k_norm_w = np.asarray(k_norm_w, dtype=np.float32)

    use_bias = bool(np.any(qkv_b != 0.0))
    key = use_bias
    if key not in _CACHE:
        _CACHE[key] = build_nc(use_bias)
    nc = _CACHE[key]
    FC = 7 if use_bias else 6

    cosq, sinq = _rope_tables(q_norm_w)
    cosk, sink = _rope_tables(k_norm_w)
    cost = np.concatenate([cosq, cosk], axis=1)  # [N, 128]
    sint = np.concatenate([sinq, sink], axis=1)

    bf16 = ml_dtypes.bfloat16
    in_maps = []
    for core in range(8):
        b, hg = core // 2, core % 2
        h0 = hg * HPC
        cols = slice(h0 * HD, (h0 + HPC) * HD)
        xt = np.ascontiguousarray(x[b].T)                       # [768, N]
        wq = qkv_w[:, cols]
        wk = qkv_w[:, D:][:, cols]
        wv = qkv_w[:, 2 * D:][:, cols]
        if use_bias:
            pad = np.zeros((128, N), np.float32)
            pad[0, :] = 1.0
            xt = np.concatenate([xt, pad], axis=0)
            wpad = np.zeros((128, HPC * HD), np.float32)
            wqb = np.concatenate([wq, wpad], axis=0)
            wkb = np.concatenate([wk, wpad], axis=0)
            wvb = np.concatenate([wv, wpad], axis=0)
            wqb[D, :] = qkv_b[cols]
            wkb[D, :] = qkv_b[D:][cols]
            wvb[D, :] = qkv_b[2 * D:][cols]
            wq, wk, wv = wqb, wkb, wvb
        wo = proj_w[h0 * HD:(h0 + HPC) * HD, :]
        im = {
            "xt": xt.astype(bf16),
            "wq": np.ascontiguousarray(wq).astype(bf16),
            "wk": np.ascontiguousarray(wk).astype(bf16),
            "wv": np.ascontiguousarray(wv).astype(bf16),
            "wo": np.ascontiguousarray(wo).astype(bf16),
            "cost": cost.astype(bf16), "sint": sint.astype(bf16),
        }
        in_maps.append(im)

    res = run_bass_kernel_spmd(nc, in_maps, core_ids=list(range(8)),
                               trace=_trace or KERNEL_TRACE)
    kernel._last = res

    y = np.empty((B, N, D), dtype=np.float32)
    for b in range(B):
        y[b] = res.results[2 * b]["y"] + res.results[2 * b + 1]["y"] + proj_b[None, :]
    return y
